# revision 31
# baseline (speedup 1.0000x reference)
"""Trainium2 Bass kernel for nn_FDConv (per-sample frequency-domain-synthesized
3x3 grouped conv).

Strategy (data-parallel over batch, 1 sample per NeuronCore):
  - host: permute dft_weight into dense half-spectrum layout (pure gather),
    precompute DFT basis matrices as constants, stage x as bf16 in a padded
    parity-split layout so every load descriptor is one 8KB contiguous run.
  - device per core:
      att = sigmoid(logits)                      (0.5 att scale folded in basis)
      GT  = (att-mixed spectrum)^T . [C | S]     (stage-1 iFFT along axis 0 via
                                                  PE matmuls; kernel-mixing att
                                                  contraction folded in)
      T[which,dx] = A_dx^T . GT_re - B_dx^T . GT_im  (stage-2 irfft + reshape to
                                                  six 128x128 conv weight mats)
      conv: x bf16 with even rows on partitions 0-63 and odd rows on 64-127;
            output row pairs (2u+1, 2u+2) computed as six K=128/M=128/N=256
            matmuls (T1_dx on slot u, T2_dx on slot u+1) accumulated in PSUM.
  - outputs copied PSUM->SBUF (DVE/ACT) and DMA'd back fp32 via SWDGE.
"""

import numpy as np
import ml_dtypes

import concourse.bass as bass
import concourse.bacc as bacc
import concourse.tile as tile
import concourse.mybir as mybir
from concourse.bass_utils import run_bass_kernel_spmd

F32 = mybir.dt.float32
BF16 = mybir.dt.bfloat16

B, CIN, COUT, KS = 8, 64, 64, 3
H, W = 256, 256
KNUM = 4
D1, D2 = COUT * KS, CIN * KS          # 192, 192
D2R = D2 // 2 + 1                     # 97
NF = D1 * D2R                         # 18624

NPAIR = 128          # output row pairs (2u+1, 2u+2), u = -1..127
SLOT = W + 2         # 258: [pad, 256 cols, pad] per row-slot
CHS = 16             # slots per x chunk
NCH = NPAIR // CHS   # 8 chunks
GRP = 16             # pairs per store group
BLK = 4              # pairs per PE block

# constant-pack column offsets (fp32, 97 partitions)
_O_DRE = 0
_O_DIM = 8 * D2R              # 776
_O_CB = 2 * 8 * D2R           # 1552
_O_SB = _O_CB + 2 * D1        # 1936
_PACKW = _O_SB + 2 * D1       # 2320


def _host_constants():
    fh = np.fft.fftfreq(D1)
    fw = np.fft.rfftfreq(D2)
    dist = np.sqrt(fh[:, None] ** 2 + fw[None, :] ** 2)
    idx = np.argsort(dist.ravel(), kind='stable')
    FH = (idx // D2R).astype(np.int64)
    FW = (idx % D2R).astype(np.int64)
    perm = FH * D2R + FW
    inv = np.empty(NF, dtype=np.int64)
    inv[perm] = np.arange(NF)

    hh = np.arange(D1)
    ang = 2.0 * np.pi * np.outer(hh, hh) / D1
    # att scale 2/KNUM = 0.5 folded into the stage-1 basis
    Cb = (np.cos(ang) * (0.5 / D1)).astype(np.float32)
    Sb = (np.sin(ang) * (0.5 / D1)).astype(np.float32)
    C2 = np.concatenate([Cb[:96], Cb[96:]], axis=1)           # [96, 384]
    S2 = np.concatenate([Sb[:96], Sb[96:]], axis=1)           # [96, 384]

    w_ = np.arange(D2R)
    n_ = np.arange(D2)
    alpha = np.full(D2R, 2.0); alpha[0] = 1.0; alpha[-1] = 1.0
    beta = np.full(D2R, 2.0); beta[0] = 0.0; beta[-1] = 0.0
    ang2 = 2.0 * np.pi * np.outer(w_, n_) / D2
    A = (alpha[:, None] * np.cos(ang2) / D2).astype(np.float32)   # [97, 192]
    Bm = (beta[:, None] * np.sin(ang2) / D2).astype(np.float32)
    ab = np.concatenate(
        [A[:, dx::3] for dx in range(3)] + [-Bm[:, dx::3] for dx in range(3)],
        axis=1,
    ).astype(ml_dtypes.bfloat16)                               # [97, 384]
    return inv, C2, S2, ab


_INV, _C2, _S2, _AB = _host_constants()

# (which, dx) order used in the conv weight loop; t_sb index = 2*dx + which
_WSEQ = [(0, 0), (1, 0), (0, 1), (1, 1), (0, 2), (1, 2)]
# valid quadrants (J, r, c0) per `which` (c0 = J - r + 2*which)
_QUADS = {
    0: [(0, 0, 0), (1, 0, 1), (1, 1, 0)],
    1: [(0, 0, 2), (0, 1, 1), (1, 1, 2)],
}
_ZQUAD = {0: (0, 1), 1: (1, 0)}  # zero quadrant (J, r)


def _emit_kernel(tc):
    nc = tc.nc
    from contextlib import ExitStack

    # x: [parity, cin, slot, 258] bf16, host-padded (col 0 and 257 are zeros)
    x_in = nc.dram_tensor("x_in", [2, CIN, NPAIR, SLOT], BF16,
                          kind="ExternalInput").ap()
    lg_in = nc.dram_tensor("lg_in", [1, KNUM], F32, kind="ExternalInput").ap()
    pk_in = nc.dram_tensor("pk_in", [D2R, _PACKW], F32, kind="ExternalInput").ap()
    ab_in = nc.dram_tensor("ab_in", [D2R, 6 * 64], BF16, kind="ExternalInput").ap()
    # device-side output layout: plane r=0 slot s = row 2s-1, plane r=1 slot
    # s = row 2s. Keeps every store descriptor contiguous (multi-KB) per
    # partition; host reassembles rows.
    out = nc.dram_tensor("out", [2, COUT, NPAIR + 1, W], F32,
                         kind="ExternalOutput").ap()

    with ExitStack() as ctx:
        cpool = ctx.enter_context(tc.tile_pool(name="cpool", bufs=1))
        xbpool = ctx.enter_context(tc.tile_pool(name="xbpool", bufs=8))
        spool = ctx.enter_context(tc.tile_pool(name="spool", bufs=3))

        # ---- small input loads (HWDGE sync queue)
        l_sb = cpool.tile([1, KNUM], F32, name="l_sb")
        nc.sync.dma_start(out=l_sb[:], in_=lg_in)
        # sigmoid emitted before the ACT-ring descgen work so it isn't stuck
        # behind blocking dma_start instructions on the ACT stream
        sig_sb = cpool.tile([1, KNUM], F32, name="sig_sb")
        nc.scalar.activation(sig_sb[:], l_sb[:], mybir.ActivationFunctionType.Sigmoid)
        # one DMA lands on ONE SDMA engine (~27 GB/s) in this runtime, so the
        # constant pack is split into parallel DMAs across both HWDGE rings;
        # the attC/attS inputs (cb|sb columns) go first since they gate the
        # dft chain.
        pk_sb = cpool.tile([D2R, _PACKW], F32, name="pk_sb")
        rsplits = ((0, 25), (25, 49), (49, 73), (73, 97))
        for r0, r1 in rsplits:
            nc.sync.dma_start(out=pk_sb[r0:r1, _O_CB:_PACKW],
                              in_=pk_in[r0:r1, _O_CB:_PACKW])
        for r0, r1 in rsplits:
            nc.scalar.dma_start(out=pk_sb[r0:r1, 0:_O_CB],
                                in_=pk_in[r0:r1, 0:_O_CB])
        ab_sb = cpool.tile([D2R, 6 * 64], BF16, name="ab_sb")
        for r0, r1 in ((0, 49), (49, 97)):
            nc.scalar.dma_start(out=ab_sb[r0:r1, :], in_=ab_in[r0:r1, :])
        dre32 = pk_sb[0:96, _O_DRE:_O_DRE + 8 * D2R]
        dim32 = pk_sb[0:96, _O_DIM:_O_DIM + 8 * D2R]
        cb_sb = pk_sb[0:96, _O_CB:_O_CB + 2 * D1]
        sb_sb = pk_sb[0:96, _O_SB:_O_SB + 2 * D1]

        # ---- att broadcast setup (K=1 matmul with ones)
        ones_sb = cpool.tile([1, 128], F32, name="ones_sb")
        nc.vector.memset(ones_sb[:], 1.0)

        # ---- x chunk loads: slot t holds rows (2t, 2t+1). Split into
        # quarter-chunk DMAs alternating SWDGE/HWDGE so many SDMA engines run.
        # Chunks >=2 are delayed behind the dft chain (dep added below) so the
        # small latency-critical loads get the early HBM bandwidth.
        xch = []
        xdmas = []
        ndma = 0
        for c in range(NCH):
            xb = xbpool.tile([128, CHS * SLOT], BF16, name="xb")
            for par in range(2):
                for p0, p1 in ((0, 32), (32, 64)):
                    eng = nc.gpsimd if ndma % 2 == 0 else nc.sync
                    di = eng.dma_start(
                        out=xb[64 * par + p0: 64 * par + p1, :],
                        in_=x_in[par, p0:p1, c * CHS:(c + 1) * CHS, :])
                    if c >= 2:
                        xdmas.append(di)
                    ndma += 1
            xch.append(xb)

        def slot_rhs(s, dx):
            c, loc = s // CHS, s % CHS
            return xch[c][:, loc * SLOT + dx: loc * SLOT + dx + W]

        # ---- bf16 casts of the permuted spectrum
        dre_sb = cpool.tile([96, 8 * D2R], BF16, name="dre_sb")
        nc.vector.tensor_copy(dre_sb[:], dre32)
        dim_sb = cpool.tile([96, 8 * D2R], BF16, name="dim_sb")
        nc.vector.tensor_copy(dim_sb[:], dim32)
        dimneg_sb = cpool.tile([96, 8 * D2R], BF16, name="dimneg_sb")
        nc.vector.tensor_scalar_mul(dimneg_sb[:], dim32, -1.0)

        gtre_sb = cpool.tile([D2R, D1 + 2], BF16, name="gtre_sb")
        gtim_sb = cpool.tile([D2R, D1 + 2], BF16, name="gtim_sb")
        t_sb = [cpool.tile([128, 128], BF16, name=f"t_sb_{i}") for i in range(6)]

        with tc.tile_pool(name="dftps", bufs=1, space="PSUM") as dpool:
            # att broadcast: [128, 4] = ones.T @ sig
            att_ps = dpool.tile([128, KNUM], F32, name="att_ps")
            nc.tensor.matmul(att_ps[:], ones_sb[:], sig_sb[:], start=True, stop=True)
            att_sb = cpool.tile([96, KNUM], F32, name="att_sb")
            nc.vector.tensor_copy(att_sb[:], att_ps[0:96, :])

            # attC/attS on DVE (ACT is busy with descgen; chunk ck = 2k + half)
            attC = cpool.tile([96, 8 * D1], BF16, name="attC")
            attS = cpool.tile([96, 8 * D1], BF16, name="attS")
            for ck in range(8):
                k, half = ck // 2, ck % 2
                nc.vector.tensor_scalar_mul(
                    attC[:, ck * D1:(ck + 1) * D1],
                    cb_sb[:, half * D1:(half + 1) * D1],
                    att_sb[:, k:k + 1])
                nc.vector.tensor_scalar_mul(
                    attS[:, ck * D1:(ck + 1) * D1],
                    sb_sb[:, half * D1:(half + 1) * D1],
                    att_sb[:, k:k + 1])

            # ---- stage 1: GT = F^T . (att*C) etc, contraction over (k, h)
            gtre_ps = dpool.tile([D2R, D1], F32, name="gtre_ps")
            gtim_ps = dpool.tile([D2R, D1], F32, name="gtim_ps")
            for ck in range(8):
                nc.tensor.matmul(
                    gtre_ps[:], dre_sb[:, ck * D2R:(ck + 1) * D2R],
                    attC[:, ck * D1:(ck + 1) * D1],
                    start=(ck == 0), stop=False)
            for ck in range(8):
                nc.tensor.matmul(
                    gtre_ps[:], dimneg_sb[:, ck * D2R:(ck + 1) * D2R],
                    attS[:, ck * D1:(ck + 1) * D1],
                    start=False, stop=(ck == 7))
            for ck in range(8):
                nc.tensor.matmul(
                    gtim_ps[:], dre_sb[:, ck * D2R:(ck + 1) * D2R],
                    attS[:, ck * D1:(ck + 1) * D1],
                    start=(ck == 0), stop=False)
            for ck in range(8):
                nc.tensor.matmul(
                    gtim_ps[:], dim_sb[:, ck * D2R:(ck + 1) * D2R],
                    attC[:, ck * D1:(ck + 1) * D1],
                    start=False, stop=(ck == 7))
            nc.vector.tensor_copy(gtre_sb[:, 0:D1], gtre_ps[:])
            nc.vector.tensor_copy(gtim_sb[:, 0:D1], gtim_ps[:])

            # ---- stage 2: six conv weight matrices T[(ci,j),(co,r)]
            def gview(g, c0):
                return g[:, c0:c0 + D1].rearrange(
                    "w (co th) -> w co th", th=3)[:, :, 0:1]

            t_copy = None
            for i, (which, dx) in enumerate(_WSEQ):
                t_ps = dpool.tile([128, 128], F32, name="t_ps", bufs=3)
                zj, zr = _ZQUAD[which]
                nc.vector.memset(t_ps[64 * zj:64 * zj + 64, 64 * zr:64 * zr + 64], 0.0)
                for (J, r, c0) in _QUADS[which]:
                    o = t_ps[64 * J:64 * J + 64, 64 * r:64 * r + 64]
                    nc.tensor.matmul(o, ab_sb[:, dx * 64:(dx + 1) * 64],
                                     gview(gtre_sb, c0), start=True, stop=False)
                    nc.tensor.matmul(o, ab_sb[:, (3 + dx) * 64:(4 + dx) * 64],
                                     gview(gtim_sb, c0), start=False, stop=True)
                t_copy = nc.vector.tensor_copy(t_sb[2 * dx + which][:], t_ps[:])
        # late x chunks wait for the dft chain to clear the HBM/SDMA path
        for di in xdmas:
            bass._add_dep_helper(di.ins, t_copy.ins,
                                 reason="late x chunks yield HBM to dft chain")

        # ---- conv over row pairs
        # staging groups over pair slots s = u+1 in [0, 129): big early, small
        # at the end so the last stores drain quickly
        gsizes = [16] * 7 + [8, 4, 2, 2, 1]
        gstart = np.cumsum([0] + gsizes).tolist()   # [0,16,...,112,120,124,126,128,129]

        def group_of(s):
            for gi in range(len(gsizes)):
                if s < gstart[gi + 1]:
                    return gi, s - gstart[gi]
            raise AssertionError

        with tc.tile_pool(name="convps", bufs=8, space="PSUM") as cps:
            staging = {}
            mm_cnt = {}
            mm_tot = {}

            def emit_block(us):
                tiles = {}
                for u in us:
                    tiles[u] = cps.tile([128, W], F32, name="pair_ps")
                    mm_cnt[u] = 0
                    mm_tot[u] = sum(
                        1 for wh, dx in _WSEQ
                        if (wh == 0 and u >= 0) or (wh == 1 and u <= 126))
                for wh, dx in _WSEQ:
                    for u in us:
                        if wh == 0 and u < 0:
                            continue
                        if wh == 1 and u > 126:
                            continue
                        rhs = slot_rhs(u + (0 if wh == 0 else 1), dx)
                        nc.tensor.matmul(
                            tiles[u][:], t_sb[2 * dx + wh][:], rhs,
                            start=(mm_cnt[u] == 0),
                            stop=(mm_cnt[u] == mm_tot[u] - 1),
                            skip_group_check=True)
                        mm_cnt[u] += 1
                for u in us:
                    gi, si = group_of(u + 1)
                    if gi not in staging:
                        staging[gi] = spool.tile(
                            [128, gsizes[gi] * W], F32, name="staging")
                    st = staging[gi][:, si * W:(si + 1) * W]
                    if u == -1:
                        nc.scalar.copy(st[64:128, :], tiles[u][64:128, :])
                    elif u == 127:
                        nc.scalar.copy(st[0:64, :], tiles[u][0:64, :])
                    elif u % 2 == 0:
                        nc.vector.tensor_copy(st, tiles[u][:])
                    else:
                        nc.scalar.copy(st, tiles[u][:])
                    if si == gsizes[gi] - 1:
                        emit_stores(gi)

            store_cnt = [0]

            def store_dma(dst, src, late=False, psplit=1):
                # round-robin store engines; sync only joins once its x-load
                # queue has drained
                engs = ([nc.gpsimd, nc.scalar, nc.sync] if late
                        else [nc.gpsimd, nc.scalar])
                for ps in range(psplit):
                    p0, p1 = 64 * ps // psplit, 64 * (ps + 1) // psplit
                    eng = engs[store_cnt[0] % len(engs)]
                    store_cnt[0] += 1
                    eng.dma_start(out=dst[p0:p1], in_=src[p0:p1])

            def emit_stores(gi):
                stg = staging.pop(gi)
                s0, s1 = gstart[gi], gstart[gi + 1]
                late = gi >= 4
                sv = stg.rearrange("p (g w) -> p g w", w=W)
                if gi == 0:
                    # row 0 from pair u=-1 -> plane 1, slot 0
                    store_dma(out[1, :, 0:1, :], sv[64:128, 0:1, :])
                # full pairs in this group: slots max(s0,1) .. min(s1,128)-1
                fa, fb = max(s0, 1), min(s1, 128)
                va = fa
                while va < fb:
                    vb = min(fb, va + 8)
                    G = vb - va
                    store_dma(out[0, :, va:vb, :],
                              sv[0:64, va - s0:va - s0 + G, :], late, 2)
                    store_dma(out[1, :, va:vb, :],
                              sv[64:128, va - s0:va - s0 + G, :], late, 2)
                    va = vb
                if s1 == 129:
                    # row 255 from pair u=127 -> plane 0, slot 128
                    store_dma(out[0, :, NPAIR:NPAIR + 1, :],
                              sv[0:64, 128 - s0:129 - s0, :], True, 2)

            emit_block([-1])
            for b0 in range(0, 128, BLK):
                emit_block(list(range(b0, b0 + BLK)))


_NC_CACHE = None


def _build_nc():
    global _NC_CACHE
    if _NC_CACHE is None:
        nc = bacc.Bacc("TRN2", target_bir_lowering=False, debug=False,
                       num_devices=B)
        with tile.TileContext(nc) as tc:
            _emit_kernel(tc)
        nc.compile()
        _NC_CACHE = nc
    return _NC_CACHE


def _in_maps(x, k_att_logits, dft_weight):
    x = np.asarray(x, dtype=np.float32)
    lg = np.asarray(k_att_logits, dtype=np.float32)
    dw = np.asarray(dft_weight, dtype=np.float32)

    # x -> bf16, parity-split rows, host-inserted zero pad columns
    xp = np.zeros((B, 2, CIN, NPAIR, SLOT), dtype=ml_dtypes.bfloat16)
    xv = x.reshape(B, CIN, NPAIR, 2, W).transpose(0, 3, 1, 2, 4)  # [b,j,c,t,w]
    xp[:, :, :, :, 1:1 + W] = xv.astype(ml_dtypes.bfloat16)

    # host-side gather: dense half-spectrum layout [k, h, w, c], chunk-major
    dftP = dw[:, _INV, :].reshape(KNUM, 2, 96, D2R, 2)   # [k, half, p, w, c]
    dre = dftP[..., 0].transpose(2, 0, 1, 3).reshape(96, 8 * D2R)
    dim = dftP[..., 1].transpose(2, 0, 1, 3).reshape(96, 8 * D2R)
    pk = np.zeros((D2R, _PACKW), dtype=np.float32)
    pk[0:96, _O_DRE:_O_DRE + 8 * D2R] = dre
    pk[0:96, _O_DIM:_O_DIM + 8 * D2R] = dim
    pk[0:96, _O_CB:_O_CB + 2 * D1] = _C2
    pk[0:96, _O_SB:_O_SB + 2 * D1] = _S2

    maps = []
    for b in range(B):
        maps.append({
            "x_in": np.ascontiguousarray(xp[b]),
            "lg_in": lg[b:b + 1],
            "pk_in": pk,
            "ab_in": _AB,
        })
    return maps


def _execute(x, k_att_logits, dft_weight, trace=False, **trace_kwargs):
    nc = _build_nc()
    res = run_bass_kernel_spmd(
        nc, _in_maps(x, k_att_logits, dft_weight),
        core_ids=list(range(B)), trace=trace, **trace_kwargs)
    out = np.empty((B, COUT, H, W), dtype=np.float32)
    for b in range(B):
        dev = res.results[b]["out"]      # [2, COUT, NPAIR+1, W]
        out[b, :, 1::2, :] = dev[0, :, 1:NPAIR + 1, :]
        out[b, :, 0::2, :] = dev[1, :, 0:NPAIR, :]
    return out, res


def kernel(x, k_att_logits, dft_weight):
    out, _ = _execute(x, k_att_logits, dft_weight)
    return out.astype(np.float32)


# revision 34
# speedup vs baseline: 1.0169x; 1.0169x over previous
"""Trainium2 Bass kernel for nn_FDConv (per-sample frequency-domain-synthesized
3x3 grouped conv).

Strategy (data-parallel over batch, 1 sample per NeuronCore):
  - host: permute dft_weight into dense half-spectrum layout (pure gather),
    precompute DFT basis matrices as constants, stage x as bf16 in a padded
    parity-split layout so every load descriptor is one 8KB contiguous run.
  - device per core:
      att = sigmoid(logits)                      (0.5 att scale folded in basis)
      GT  = (att-mixed spectrum)^T . [C | S]     (stage-1 iFFT along axis 0 via
                                                  PE matmuls; kernel-mixing att
                                                  contraction folded in)
      T[which,dx] = A_dx^T . GT_re - B_dx^T . GT_im  (stage-2 irfft + reshape to
                                                  six 128x128 conv weight mats)
      conv: x bf16 with even rows on partitions 0-63 and odd rows on 64-127;
            output row pairs (2u+1, 2u+2) computed as six K=128/M=128/N=256
            matmuls (T1_dx on slot u, T2_dx on slot u+1) accumulated in PSUM.
  - outputs copied PSUM->SBUF (DVE/ACT) and DMA'd back fp32 via SWDGE.
"""

import numpy as np
import ml_dtypes

import concourse.bass as bass
import concourse.bacc as bacc
import concourse.tile as tile
import concourse.mybir as mybir
from concourse.bass_utils import run_bass_kernel_spmd

F32 = mybir.dt.float32
BF16 = mybir.dt.bfloat16

B, CIN, COUT, KS = 8, 64, 64, 3
H, W = 256, 256
KNUM = 4
D1, D2 = COUT * KS, CIN * KS          # 192, 192
D2R = D2 // 2 + 1                     # 97
NF = D1 * D2R                         # 18624

NPAIR = 128          # output row pairs (2u+1, 2u+2), u = -1..127
SLOT = W + 2         # 258: [pad, 256 cols, pad] per row-slot
CHS = 16             # slots per x chunk
NCH = NPAIR // CHS   # 8 chunks
GRP = 16             # pairs per store group
BLK = 4              # pairs per PE block

# constant-pack column offsets (fp32, 97 partitions)
_O_DRE = 0
_O_DIM = 8 * D2R              # 776
_O_CB = 2 * 8 * D2R           # 1552
_O_SB = _O_CB + 2 * D1        # 1936
_PACKW = _O_SB + 2 * D1       # 2320


def _host_constants():
    fh = np.fft.fftfreq(D1)
    fw = np.fft.rfftfreq(D2)
    dist = np.sqrt(fh[:, None] ** 2 + fw[None, :] ** 2)
    idx = np.argsort(dist.ravel(), kind='stable')
    FH = (idx // D2R).astype(np.int64)
    FW = (idx % D2R).astype(np.int64)
    perm = FH * D2R + FW
    inv = np.empty(NF, dtype=np.int64)
    inv[perm] = np.arange(NF)

    hh = np.arange(D1)
    ang = 2.0 * np.pi * np.outer(hh, hh) / D1
    # att scale 2/KNUM = 0.5 folded into the stage-1 basis
    Cb = (np.cos(ang) * (0.5 / D1)).astype(np.float32)
    Sb = (np.sin(ang) * (0.5 / D1)).astype(np.float32)
    C2 = np.concatenate([Cb[:96], Cb[96:]], axis=1)           # [96, 384]
    S2 = np.concatenate([Sb[:96], Sb[96:]], axis=1)           # [96, 384]

    w_ = np.arange(D2R)
    n_ = np.arange(D2)
    alpha = np.full(D2R, 2.0); alpha[0] = 1.0; alpha[-1] = 1.0
    beta = np.full(D2R, 2.0); beta[0] = 0.0; beta[-1] = 0.0
    ang2 = 2.0 * np.pi * np.outer(w_, n_) / D2
    A = (alpha[:, None] * np.cos(ang2) / D2).astype(np.float32)   # [97, 192]
    Bm = (beta[:, None] * np.sin(ang2) / D2).astype(np.float32)
    ab = np.concatenate(
        [A[:, dx::3] for dx in range(3)] + [-Bm[:, dx::3] for dx in range(3)],
        axis=1,
    ).astype(ml_dtypes.bfloat16)                               # [97, 384]
    return inv, C2, S2, ab


_INV, _C2, _S2, _AB = _host_constants()

# (which, dx) order used in the conv weight loop; t_sb index = 2*dx + which
_WSEQ = [(0, 0), (1, 0), (0, 1), (1, 1), (0, 2), (1, 2)]
# valid quadrants (J, r, c0) per `which` (c0 = J - r + 2*which)
_QUADS = {
    0: [(0, 0, 0), (1, 0, 1), (1, 1, 0)],
    1: [(0, 0, 2), (0, 1, 1), (1, 1, 2)],
}
_ZQUAD = {0: (0, 1), 1: (1, 0)}  # zero quadrant (J, r)


def _emit_kernel(tc):
    nc = tc.nc
    from contextlib import ExitStack

    # x: [parity, cin, slot, 258] bf16, host-padded (col 0 and 257 are zeros)
    x_in = nc.dram_tensor("x_in", [2, CIN, NPAIR, SLOT], BF16,
                          kind="ExternalInput").ap()
    lg_in = nc.dram_tensor("lg_in", [1, KNUM], F32, kind="ExternalInput").ap()
    pk_in = nc.dram_tensor("pk_in", [D2R, _PACKW], F32, kind="ExternalInput").ap()
    ab_in = nc.dram_tensor("ab_in", [D2R, 6 * 64], BF16, kind="ExternalInput").ap()
    # device-side output layout: plane r=0 slot s = row 2s-1, plane r=1 slot
    # s = row 2s. Keeps every store descriptor contiguous (multi-KB) per
    # partition; host reassembles rows.
    out = nc.dram_tensor("out", [2, COUT, NPAIR + 1, W], F32,
                         kind="ExternalOutput").ap()

    with ExitStack() as ctx:
        cpool = ctx.enter_context(tc.tile_pool(name="cpool", bufs=1))
        xbpool = ctx.enter_context(tc.tile_pool(name="xbpool", bufs=8))
        spool = ctx.enter_context(tc.tile_pool(name="spool", bufs=3))

        # ---- small input loads (HWDGE sync queue)
        l_sb = cpool.tile([1, KNUM], F32, name="l_sb")
        nc.sync.dma_start(out=l_sb[:], in_=lg_in)
        # sigmoid emitted before the ACT-ring descgen work so it isn't stuck
        # behind blocking dma_start instructions on the ACT stream
        sig_sb = cpool.tile([1, KNUM], F32, name="sig_sb")
        nc.scalar.activation(sig_sb[:], l_sb[:], mybir.ActivationFunctionType.Sigmoid)
        # one DMA lands on ONE SDMA engine (~27 GB/s) with ring depths of ~8
        # (SWDGE) / ~2 (each HWDGE ring), so the latency-critical constant
        # pack is split into 8 row slices spread over all three rings.
        pk_sb = cpool.tile([D2R, _PACKW], F32, name="pk_sb")
        redges = (0, 13, 25, 37, 49, 61, 73, 85, 97)
        pk_engs = [nc.gpsimd, nc.gpsimd, nc.gpsimd, nc.gpsimd,
                   nc.sync, nc.sync, nc.scalar, nc.scalar]
        for k in range(8):
            r0, r1 = redges[k], redges[k + 1]
            pk_engs[k].dma_start(out=pk_sb[r0:r1, :], in_=pk_in[r0:r1, :])
        ab_sb = cpool.tile([D2R, 6 * 64], BF16, name="ab_sb")
        for r0, r1 in ((0, 49), (49, 97)):
            nc.scalar.dma_start(out=ab_sb[r0:r1, :], in_=ab_in[r0:r1, :])
        dre32 = pk_sb[0:96, _O_DRE:_O_DRE + 8 * D2R]
        dim32 = pk_sb[0:96, _O_DIM:_O_DIM + 8 * D2R]
        cb_sb = pk_sb[0:96, _O_CB:_O_CB + 2 * D1]
        sb_sb = pk_sb[0:96, _O_SB:_O_SB + 2 * D1]

        # ---- att broadcast setup (K=1 matmul with ones)
        ones_sb = cpool.tile([1, 128], F32, name="ones_sb")
        nc.vector.memset(ones_sb[:], 1.0)

        # ---- x chunk loads: slot t holds rows (2t, 2t+1). Split into
        # quarter-chunk DMAs alternating SWDGE/HWDGE so many SDMA engines run.
        # Chunks >=2 are delayed behind the dft chain (dep added below) so the
        # small latency-critical loads get the early HBM bandwidth.
        xch = []
        xdmas = []
        early_engs = [nc.gpsimd, nc.sync, nc.gpsimd, nc.scalar,
                      nc.gpsimd, nc.sync, nc.gpsimd, nc.scalar]
        ndma = 0
        for c in range(NCH):
            xb = xbpool.tile([128, CHS * SLOT], BF16, name="xb")
            for par in range(2):
                for p0, p1 in ((0, 32), (32, 64)):
                    eng = early_engs[ndma % 8] if c < 2 else nc.gpsimd
                    di = eng.dma_start(
                        out=xb[64 * par + p0: 64 * par + p1, :],
                        in_=x_in[par, p0:p1, c * CHS:(c + 1) * CHS, :])
                    if c >= 2:
                        xdmas.append(di)
                    ndma += 1
            xch.append(xb)

        def slot_rhs(s, dx):
            c, loc = s // CHS, s % CHS
            return xch[c][:, loc * SLOT + dx: loc * SLOT + dx + W]

        # ---- bf16 casts of the permuted spectrum
        dre_sb = cpool.tile([96, 8 * D2R], BF16, name="dre_sb")
        nc.vector.tensor_copy(dre_sb[:], dre32)
        dim_sb = cpool.tile([96, 8 * D2R], BF16, name="dim_sb")
        nc.vector.tensor_copy(dim_sb[:], dim32)
        dimneg_sb = cpool.tile([96, 8 * D2R], BF16, name="dimneg_sb")
        nc.vector.tensor_scalar_mul(dimneg_sb[:], dim32, -1.0)

        gtre_sb = cpool.tile([D2R, D1 + 2], BF16, name="gtre_sb")
        gtim_sb = cpool.tile([D2R, D1 + 2], BF16, name="gtim_sb")
        t_sb = [cpool.tile([128, 128], BF16, name=f"t_sb_{i}") for i in range(6)]

        with tc.tile_pool(name="dftps", bufs=1, space="PSUM") as dpool:
            # att broadcast: [128, 4] = ones.T @ sig
            att_ps = dpool.tile([128, KNUM], F32, name="att_ps")
            nc.tensor.matmul(att_ps[:], ones_sb[:], sig_sb[:], start=True, stop=True)
            att_sb = cpool.tile([96, KNUM], F32, name="att_sb")
            nc.vector.tensor_copy(att_sb[:], att_ps[0:96, :])

            # attC/attS on DVE (ACT is busy with descgen; chunk ck = 2k + half)
            attC = cpool.tile([96, 8 * D1], BF16, name="attC")
            attS = cpool.tile([96, 8 * D1], BF16, name="attS")
            for ck in range(8):
                k, half = ck // 2, ck % 2
                nc.vector.tensor_scalar_mul(
                    attC[:, ck * D1:(ck + 1) * D1],
                    cb_sb[:, half * D1:(half + 1) * D1],
                    att_sb[:, k:k + 1])
                nc.vector.tensor_scalar_mul(
                    attS[:, ck * D1:(ck + 1) * D1],
                    sb_sb[:, half * D1:(half + 1) * D1],
                    att_sb[:, k:k + 1])

            # ---- stage 1: GT = F^T . (att*C) etc, contraction over (k, h)
            gtre_ps = dpool.tile([D2R, D1], F32, name="gtre_ps")
            gtim_ps = dpool.tile([D2R, D1], F32, name="gtim_ps")
            for ck in range(8):
                nc.tensor.matmul(
                    gtre_ps[:], dre_sb[:, ck * D2R:(ck + 1) * D2R],
                    attC[:, ck * D1:(ck + 1) * D1],
                    start=(ck == 0), stop=False)
            for ck in range(8):
                nc.tensor.matmul(
                    gtre_ps[:], dimneg_sb[:, ck * D2R:(ck + 1) * D2R],
                    attS[:, ck * D1:(ck + 1) * D1],
                    start=False, stop=(ck == 7))
            for ck in range(8):
                nc.tensor.matmul(
                    gtim_ps[:], dre_sb[:, ck * D2R:(ck + 1) * D2R],
                    attS[:, ck * D1:(ck + 1) * D1],
                    start=(ck == 0), stop=False)
            for ck in range(8):
                nc.tensor.matmul(
                    gtim_ps[:], dim_sb[:, ck * D2R:(ck + 1) * D2R],
                    attC[:, ck * D1:(ck + 1) * D1],
                    start=False, stop=(ck == 7))
            nc.vector.tensor_copy(gtre_sb[:, 0:D1], gtre_ps[:])
            nc.vector.tensor_copy(gtim_sb[:, 0:D1], gtim_ps[:])

            # ---- stage 2: six conv weight matrices T[(ci,j),(co,r)]
            def gview(g, c0):
                return g[:, c0:c0 + D1].rearrange(
                    "w (co th) -> w co th", th=3)[:, :, 0:1]

            t_copy = None
            for i, (which, dx) in enumerate(_WSEQ):
                t_ps = dpool.tile([128, 128], F32, name="t_ps", bufs=3)
                zj, zr = _ZQUAD[which]
                nc.vector.memset(t_ps[64 * zj:64 * zj + 64, 64 * zr:64 * zr + 64], 0.0)
                for (J, r, c0) in _QUADS[which]:
                    o = t_ps[64 * J:64 * J + 64, 64 * r:64 * r + 64]
                    nc.tensor.matmul(o, ab_sb[:, dx * 64:(dx + 1) * 64],
                                     gview(gtre_sb, c0), start=True, stop=False)
                    nc.tensor.matmul(o, ab_sb[:, (3 + dx) * 64:(4 + dx) * 64],
                                     gview(gtim_sb, c0), start=False, stop=True)
                t_copy = nc.vector.tensor_copy(t_sb[2 * dx + which][:], t_ps[:])
        # late x chunks wait for the dft chain to clear the HBM/SDMA path
        for di in xdmas:
            bass._add_dep_helper(di.ins, t_copy.ins,
                                 reason="late x chunks yield HBM to dft chain")

        # ---- conv over row pairs
        # staging groups over pair slots s = u+1 in [0, 129): big early, small
        # at the end so the last stores drain quickly
        gsizes = [16] * 7 + [8, 4, 2, 2, 1]
        gstart = np.cumsum([0] + gsizes).tolist()   # [0,16,...,112,120,124,126,128,129]

        def group_of(s):
            for gi in range(len(gsizes)):
                if s < gstart[gi + 1]:
                    return gi, s - gstart[gi]
            raise AssertionError

        with tc.tile_pool(name="convps", bufs=8, space="PSUM") as cps:
            staging = {}
            mm_cnt = {}
            mm_tot = {}

            def emit_block(us):
                tiles = {}
                for u in us:
                    tiles[u] = cps.tile([128, W], F32, name="pair_ps")
                    mm_cnt[u] = 0
                    mm_tot[u] = sum(
                        1 for wh, dx in _WSEQ
                        if (wh == 0 and u >= 0) or (wh == 1 and u <= 126))
                for wh, dx in _WSEQ:
                    for u in us:
                        if wh == 0 and u < 0:
                            continue
                        if wh == 1 and u > 126:
                            continue
                        rhs = slot_rhs(u + (0 if wh == 0 else 1), dx)
                        nc.tensor.matmul(
                            tiles[u][:], t_sb[2 * dx + wh][:], rhs,
                            start=(mm_cnt[u] == 0),
                            stop=(mm_cnt[u] == mm_tot[u] - 1),
                            skip_group_check=True)
                        mm_cnt[u] += 1
                for u in us:
                    gi, si = group_of(u + 1)
                    if gi not in staging:
                        staging[gi] = spool.tile(
                            [128, gsizes[gi] * W], F32, name="staging")
                    st = staging[gi][:, si * W:(si + 1) * W]
                    if u == -1:
                        nc.scalar.copy(st[64:128, :], tiles[u][64:128, :])
                    elif u == 127:
                        nc.scalar.copy(st[0:64, :], tiles[u][0:64, :])
                    elif u % 2 == 0:
                        nc.vector.tensor_copy(st, tiles[u][:])
                    else:
                        nc.scalar.copy(st, tiles[u][:])
                    if si == gsizes[gi] - 1:
                        emit_stores(gi)

            store_cnt = [0]

            def store_dma(dst, src, late=False, psplit=1):
                # round-robin store engines; sync only joins once its x-load
                # queue has drained
                engs = ([nc.gpsimd, nc.scalar, nc.sync] if late
                        else [nc.gpsimd, nc.scalar])
                for ps in range(psplit):
                    p0, p1 = 64 * ps // psplit, 64 * (ps + 1) // psplit
                    eng = engs[store_cnt[0] % len(engs)]
                    store_cnt[0] += 1
                    eng.dma_start(out=dst[p0:p1], in_=src[p0:p1])

            def emit_stores(gi):
                stg = staging.pop(gi)
                s0, s1 = gstart[gi], gstart[gi + 1]
                late = gi >= 4
                sv = stg.rearrange("p (g w) -> p g w", w=W)
                if gi == 0:
                    # row 0 from pair u=-1 -> plane 1, slot 0
                    store_dma(out[1, :, 0:1, :], sv[64:128, 0:1, :])
                # full pairs in this group: slots max(s0,1) .. min(s1,128)-1
                fa, fb = max(s0, 1), min(s1, 128)
                run = 8 if gi < 7 else 4
                va = fa
                while va < fb:
                    vb = min(fb, va + run)
                    G = vb - va
                    store_dma(out[0, :, va:vb, :],
                              sv[0:64, va - s0:va - s0 + G, :], late, 2)
                    store_dma(out[1, :, va:vb, :],
                              sv[64:128, va - s0:va - s0 + G, :], late, 2)
                    va = vb
                if s1 == 129:
                    # row 255 from pair u=127 -> plane 0, slot 128
                    store_dma(out[0, :, NPAIR:NPAIR + 1, :],
                              sv[0:64, 128 - s0:129 - s0, :], True, 2)

            emit_block([-1])
            for b0 in range(0, 128, BLK):
                emit_block(list(range(b0, b0 + BLK)))


_NC_CACHE = None


def _build_nc():
    global _NC_CACHE
    if _NC_CACHE is None:
        nc = bacc.Bacc("TRN2", target_bir_lowering=False, debug=False,
                       num_devices=B)
        with tile.TileContext(nc) as tc:
            _emit_kernel(tc)
        nc.compile()
        _NC_CACHE = nc
    return _NC_CACHE


def _in_maps(x, k_att_logits, dft_weight):
    x = np.asarray(x, dtype=np.float32)
    lg = np.asarray(k_att_logits, dtype=np.float32)
    dw = np.asarray(dft_weight, dtype=np.float32)

    # x -> bf16, parity-split rows, host-inserted zero pad columns
    xp = np.zeros((B, 2, CIN, NPAIR, SLOT), dtype=ml_dtypes.bfloat16)
    xv = x.reshape(B, CIN, NPAIR, 2, W).transpose(0, 3, 1, 2, 4)  # [b,j,c,t,w]
    xp[:, :, :, :, 1:1 + W] = xv.astype(ml_dtypes.bfloat16)

    # host-side gather: dense half-spectrum layout [k, h, w, c], chunk-major
    dftP = dw[:, _INV, :].reshape(KNUM, 2, 96, D2R, 2)   # [k, half, p, w, c]
    dre = dftP[..., 0].transpose(2, 0, 1, 3).reshape(96, 8 * D2R)
    dim = dftP[..., 1].transpose(2, 0, 1, 3).reshape(96, 8 * D2R)
    pk = np.zeros((D2R, _PACKW), dtype=np.float32)
    pk[0:96, _O_DRE:_O_DRE + 8 * D2R] = dre
    pk[0:96, _O_DIM:_O_DIM + 8 * D2R] = dim
    pk[0:96, _O_CB:_O_CB + 2 * D1] = _C2
    pk[0:96, _O_SB:_O_SB + 2 * D1] = _S2

    maps = []
    for b in range(B):
        maps.append({
            "x_in": np.ascontiguousarray(xp[b]),
            "lg_in": lg[b:b + 1],
            "pk_in": pk,
            "ab_in": _AB,
        })
    return maps


def _execute(x, k_att_logits, dft_weight, trace=False, **trace_kwargs):
    nc = _build_nc()
    res = run_bass_kernel_spmd(
        nc, _in_maps(x, k_att_logits, dft_weight),
        core_ids=list(range(B)), trace=trace, **trace_kwargs)
    out = np.empty((B, COUT, H, W), dtype=np.float32)
    for b in range(B):
        dev = res.results[b]["out"]      # [2, COUT, NPAIR+1, W]
        out[b, :, 1::2, :] = dev[0, :, 1:NPAIR + 1, :]
        out[b, :, 0::2, :] = dev[1, :, 0:NPAIR, :]
    return out, res


def kernel(x, k_att_logits, dft_weight):
    out, _ = _execute(x, k_att_logits, dft_weight)
    return out.astype(np.float32)


# revision 36
# speedup vs baseline: 1.0230x; 1.0061x over previous
"""Trainium2 Bass kernel for nn_FDConv (per-sample frequency-domain-synthesized
3x3 grouped conv).

Strategy (data-parallel over batch, 1 sample per NeuronCore):
  - host: permute dft_weight into dense half-spectrum layout (pure gather),
    precompute DFT basis matrices as constants, stage x as bf16 in a padded
    parity-split layout so every load descriptor is one 8KB contiguous run.
  - device per core:
      att = sigmoid(logits)                      (0.5 att scale folded in basis)
      GT  = (att-mixed spectrum)^T . [C | S]     (stage-1 iFFT along axis 0 via
                                                  PE matmuls; kernel-mixing att
                                                  contraction folded in)
      T[which,dx] = A_dx^T . GT_re - B_dx^T . GT_im  (stage-2 irfft + reshape to
                                                  six 128x128 conv weight mats)
      conv: x bf16 with even rows on partitions 0-63 and odd rows on 64-127;
            output row pairs (2u+1, 2u+2) computed as six K=128/M=128/N=256
            matmuls (T1_dx on slot u, T2_dx on slot u+1) accumulated in PSUM.
  - outputs copied PSUM->SBUF (DVE/ACT) and DMA'd back fp32 via SWDGE.
"""

import numpy as np
import ml_dtypes

import concourse.bass as bass
import concourse.bacc as bacc
import concourse.tile as tile
import concourse.mybir as mybir
from concourse.bass_utils import run_bass_kernel_spmd

F32 = mybir.dt.float32
BF16 = mybir.dt.bfloat16

B, CIN, COUT, KS = 8, 64, 64, 3
H, W = 256, 256
KNUM = 4
D1, D2 = COUT * KS, CIN * KS          # 192, 192
D2R = D2 // 2 + 1                     # 97
NF = D1 * D2R                         # 18624

NPAIR = 128          # output row pairs (2u+1, 2u+2), u = -1..127
SLOT = W + 2         # 258: [pad, 256 cols, pad] per row-slot
CHS = 16             # slots per x chunk
NCH = NPAIR // CHS   # 8 chunks
GRP = 16             # pairs per store group
BLK = 4              # pairs per PE block

# constant-pack column offsets (fp32, 97 partitions)
_O_DRE = 0
_O_DIM = 8 * D2R              # 776
_O_CB = 2 * 8 * D2R           # 1552
_O_SB = _O_CB + 2 * D1        # 1936
_PACKW = _O_SB + 2 * D1       # 2320


def _host_constants():
    fh = np.fft.fftfreq(D1)
    fw = np.fft.rfftfreq(D2)
    dist = np.sqrt(fh[:, None] ** 2 + fw[None, :] ** 2)
    idx = np.argsort(dist.ravel(), kind='stable')
    FH = (idx // D2R).astype(np.int64)
    FW = (idx % D2R).astype(np.int64)
    perm = FH * D2R + FW
    inv = np.empty(NF, dtype=np.int64)
    inv[perm] = np.arange(NF)

    hh = np.arange(D1)
    ang = 2.0 * np.pi * np.outer(hh, hh) / D1
    # att scale 2/KNUM = 0.5 folded into the stage-1 basis
    Cb = (np.cos(ang) * (0.5 / D1)).astype(np.float32)
    Sb = (np.sin(ang) * (0.5 / D1)).astype(np.float32)
    C2 = np.concatenate([Cb[:96], Cb[96:]], axis=1)           # [96, 384]
    S2 = np.concatenate([Sb[:96], Sb[96:]], axis=1)           # [96, 384]

    w_ = np.arange(D2R)
    n_ = np.arange(D2)
    alpha = np.full(D2R, 2.0); alpha[0] = 1.0; alpha[-1] = 1.0
    beta = np.full(D2R, 2.0); beta[0] = 0.0; beta[-1] = 0.0
    ang2 = 2.0 * np.pi * np.outer(w_, n_) / D2
    A = (alpha[:, None] * np.cos(ang2) / D2).astype(np.float32)   # [97, 192]
    Bm = (beta[:, None] * np.sin(ang2) / D2).astype(np.float32)
    ab = np.concatenate(
        [A[:, dx::3] for dx in range(3)] + [-Bm[:, dx::3] for dx in range(3)],
        axis=1,
    ).astype(ml_dtypes.bfloat16)                               # [97, 384]
    return inv, C2, S2, ab


_INV, _C2, _S2, _AB = _host_constants()

# (which, dx) order used in the conv weight loop; t_sb index = 2*dx + which
_WSEQ = [(0, 0), (1, 0), (0, 1), (1, 1), (0, 2), (1, 2)]
# valid quadrants (J, r, c0) per `which` (c0 = J - r + 2*which)
_QUADS = {
    0: [(0, 0, 0), (1, 0, 1), (1, 1, 0)],
    1: [(0, 0, 2), (0, 1, 1), (1, 1, 2)],
}
_ZQUAD = {0: (0, 1), 1: (1, 0)}  # zero quadrant (J, r)


def _emit_kernel(tc):
    nc = tc.nc
    from contextlib import ExitStack

    # x: [parity, cin, slot, 258] bf16, host-padded (col 0 and 257 are zeros)
    x_in = nc.dram_tensor("x_in", [2, CIN, NPAIR, SLOT], BF16,
                          kind="ExternalInput").ap()
    lg_in = nc.dram_tensor("lg_in", [1, KNUM], F32, kind="ExternalInput").ap()
    pk_in = nc.dram_tensor("pk_in", [D2R, _PACKW], F32, kind="ExternalInput").ap()
    ab_in = nc.dram_tensor("ab_in", [D2R, 6 * 64], BF16, kind="ExternalInput").ap()
    # device-side output layout: plane r=0 slot s = row 2s-1, plane r=1 slot
    # s = row 2s. Keeps every store descriptor contiguous (multi-KB) per
    # partition; host reassembles rows.
    out = nc.dram_tensor("out", [2, COUT, NPAIR + 1, W], F32,
                         kind="ExternalOutput").ap()

    with ExitStack() as ctx:
        cpool = ctx.enter_context(tc.tile_pool(name="cpool", bufs=1))
        xbpool = ctx.enter_context(tc.tile_pool(name="xbpool", bufs=8))
        spool = ctx.enter_context(tc.tile_pool(name="spool", bufs=3))

        # ---- small input loads (HWDGE sync queue)
        l_sb = cpool.tile([1, KNUM], F32, name="l_sb")
        nc.sync.dma_start(out=l_sb[:], in_=lg_in)
        # sigmoid emitted before the ACT-ring descgen work so it isn't stuck
        # behind blocking dma_start instructions on the ACT stream
        sig_sb = cpool.tile([1, KNUM], F32, name="sig_sb")
        nc.scalar.activation(sig_sb[:], l_sb[:], mybir.ActivationFunctionType.Sigmoid)
        # one DMA lands on ONE SDMA engine (~27 GB/s) with ring depths of ~8
        # (SWDGE) / ~2 (each HWDGE ring), so the latency-critical constant
        # pack is split into 8 row slices spread over all three rings.
        pk_sb = cpool.tile([D2R, _PACKW], F32, name="pk_sb")
        redges = (0, 13, 25, 37, 49, 61, 73, 85, 97)
        pk_engs = [nc.gpsimd, nc.gpsimd, nc.gpsimd, nc.gpsimd,
                   nc.sync, nc.sync, nc.scalar, nc.scalar]
        for k in range(8):
            r0, r1 = redges[k], redges[k + 1]
            pk_engs[k].dma_start(out=pk_sb[r0:r1, :], in_=pk_in[r0:r1, :])
        ab_sb = cpool.tile([D2R, 6 * 64], BF16, name="ab_sb")
        for r0, r1 in ((0, 49), (49, 97)):
            nc.scalar.dma_start(out=ab_sb[r0:r1, :], in_=ab_in[r0:r1, :])
        dre32 = pk_sb[0:96, _O_DRE:_O_DRE + 8 * D2R]
        dim32 = pk_sb[0:96, _O_DIM:_O_DIM + 8 * D2R]
        cb_sb = pk_sb[0:96, _O_CB:_O_CB + 2 * D1]
        sb_sb = pk_sb[0:96, _O_SB:_O_SB + 2 * D1]

        # ---- att broadcast setup (K=1 matmul with ones)
        ones_sb = cpool.tile([1, 128], F32, name="ones_sb")
        nc.vector.memset(ones_sb[:], 1.0)

        # ---- x chunk loads: slot t holds rows (2t, 2t+1). Split into
        # quarter-chunk DMAs alternating SWDGE/HWDGE so many SDMA engines run.
        # Chunks >=2 are delayed behind the dft chain (dep added below) so the
        # small latency-critical loads get the early HBM bandwidth.
        xch = []
        xdmas = []
        early_engs = [nc.gpsimd, nc.sync, nc.gpsimd, nc.scalar,
                      nc.gpsimd, nc.sync, nc.gpsimd, nc.scalar]
        ndma = 0
        for c in range(NCH):
            xb = xbpool.tile([128, CHS * SLOT], BF16, name="xb")
            for par in range(2):
                for p0, p1 in ((0, 32), (32, 64)):
                    eng = early_engs[ndma % 8] if c < 2 else nc.gpsimd
                    di = eng.dma_start(
                        out=xb[64 * par + p0: 64 * par + p1, :],
                        in_=x_in[par, p0:p1, c * CHS:(c + 1) * CHS, :])
                    if c >= 2:
                        xdmas.append(di)
                    ndma += 1
            xch.append(xb)

        def slot_rhs(s, dx):
            c, loc = s // CHS, s % CHS
            return xch[c][:, loc * SLOT + dx: loc * SLOT + dx + W]

        # ---- bf16 casts of the permuted spectrum
        dre_sb = cpool.tile([96, 8 * D2R], BF16, name="dre_sb")
        nc.vector.tensor_copy(dre_sb[:], dre32)
        dim_sb = cpool.tile([96, 8 * D2R], BF16, name="dim_sb")
        nc.vector.tensor_copy(dim_sb[:], dim32)
        dimneg_sb = cpool.tile([96, 8 * D2R], BF16, name="dimneg_sb")
        nc.vector.tensor_scalar_mul(dimneg_sb[:], dim32, -1.0)

        gtre_sb = cpool.tile([D2R, D1 + 2], BF16, name="gtre_sb")
        gtim_sb = cpool.tile([D2R, D1 + 2], BF16, name="gtim_sb")
        t_sb = [cpool.tile([128, 128], BF16, name=f"t_sb_{i}") for i in range(6)]

        with tc.tile_pool(name="dftps", bufs=1, space="PSUM") as dpool:
            # att broadcast: [128, 4] = ones.T @ sig
            att_ps = dpool.tile([128, KNUM], F32, name="att_ps")
            nc.tensor.matmul(att_ps[:], ones_sb[:], sig_sb[:], start=True, stop=True)
            att_sb = cpool.tile([96, KNUM], F32, name="att_sb")
            nc.vector.tensor_copy(att_sb[:], att_ps[0:96, :])

            # attC/attS on DVE (ACT is busy with descgen; chunk ck = 2k + half)
            attC = cpool.tile([96, 8 * D1], BF16, name="attC")
            attS = cpool.tile([96, 8 * D1], BF16, name="attS")
            for ck in range(8):
                k, half = ck // 2, ck % 2
                nc.vector.tensor_scalar_mul(
                    attC[:, ck * D1:(ck + 1) * D1],
                    cb_sb[:, half * D1:(half + 1) * D1],
                    att_sb[:, k:k + 1])
                nc.vector.tensor_scalar_mul(
                    attS[:, ck * D1:(ck + 1) * D1],
                    sb_sb[:, half * D1:(half + 1) * D1],
                    att_sb[:, k:k + 1])

            # ---- stage 1: GT = F^T . (att*C) etc, contraction over (k, h)
            gtre_ps = dpool.tile([D2R, D1], F32, name="gtre_ps")
            gtim_ps = dpool.tile([D2R, D1], F32, name="gtim_ps")
            for ck in range(8):
                nc.tensor.matmul(
                    gtre_ps[:], dre_sb[:, ck * D2R:(ck + 1) * D2R],
                    attC[:, ck * D1:(ck + 1) * D1],
                    start=(ck == 0), stop=False)
            for ck in range(8):
                nc.tensor.matmul(
                    gtre_ps[:], dimneg_sb[:, ck * D2R:(ck + 1) * D2R],
                    attS[:, ck * D1:(ck + 1) * D1],
                    start=False, stop=(ck == 7))
            for ck in range(8):
                nc.tensor.matmul(
                    gtim_ps[:], dre_sb[:, ck * D2R:(ck + 1) * D2R],
                    attS[:, ck * D1:(ck + 1) * D1],
                    start=(ck == 0), stop=False)
            for ck in range(8):
                nc.tensor.matmul(
                    gtim_ps[:], dim_sb[:, ck * D2R:(ck + 1) * D2R],
                    attC[:, ck * D1:(ck + 1) * D1],
                    start=False, stop=(ck == 7))
            nc.vector.tensor_copy(gtre_sb[:, 0:D1], gtre_ps[:])
            nc.vector.tensor_copy(gtim_sb[:, 0:D1], gtim_ps[:])

            # ---- stage 2: six conv weight matrices T[(ci,j),(co,r)]
            def gview(g, c0):
                return g[:, c0:c0 + D1].rearrange(
                    "w (co th) -> w co th", th=3)[:, :, 0:1]

            t_copy = None
            for i, (which, dx) in enumerate(_WSEQ):
                t_ps = dpool.tile([128, 128], F32, name="t_ps", bufs=3)
                zj, zr = _ZQUAD[which]
                nc.vector.memset(t_ps[64 * zj:64 * zj + 64, 64 * zr:64 * zr + 64], 0.0)
                for (J, r, c0) in _QUADS[which]:
                    o = t_ps[64 * J:64 * J + 64, 64 * r:64 * r + 64]
                    nc.tensor.matmul(o, ab_sb[:, dx * 64:(dx + 1) * 64],
                                     gview(gtre_sb, c0), start=True, stop=False)
                    nc.tensor.matmul(o, ab_sb[:, (3 + dx) * 64:(4 + dx) * 64],
                                     gview(gtim_sb, c0), start=False, stop=True)
                t_copy = nc.vector.tensor_copy(t_sb[2 * dx + which][:], t_ps[:])
        # late x chunks wait for the dft chain to clear the HBM/SDMA path
        for di in xdmas:
            bass._add_dep_helper(di.ins, t_copy.ins,
                                 reason="late x chunks yield HBM to dft chain")

        # ---- conv over row pairs
        # staging groups over pair slots s = u+1 in [0, 129): big early, small
        # at the end so the last stores drain quickly
        gsizes = [16] * 7 + [8, 4, 2, 2, 1]
        gstart = np.cumsum([0] + gsizes).tolist()   # [0,16,...,112,120,124,126,128,129]

        def group_of(s):
            for gi in range(len(gsizes)):
                if s < gstart[gi + 1]:
                    return gi, s - gstart[gi]
            raise AssertionError

        with tc.tile_pool(name="convps", bufs=8, space="PSUM") as cps:
            staging = {}
            mm_cnt = {}
            mm_tot = {}

            def emit_block(us):
                tiles = {}
                for u in us:
                    tiles[u] = cps.tile([128, W], F32, name="pair_ps")
                    mm_cnt[u] = 0
                    mm_tot[u] = sum(
                        1 for wh, dx in _WSEQ
                        if (wh == 0 and u >= 0) or (wh == 1 and u <= 126))
                for wh, dx in _WSEQ:
                    for u in us:
                        if wh == 0 and u < 0:
                            continue
                        if wh == 1 and u > 126:
                            continue
                        rhs = slot_rhs(u + (0 if wh == 0 else 1), dx)
                        nc.tensor.matmul(
                            tiles[u][:], t_sb[2 * dx + wh][:], rhs,
                            start=(mm_cnt[u] == 0),
                            stop=(mm_cnt[u] == mm_tot[u] - 1),
                            skip_group_check=True)
                        mm_cnt[u] += 1
                for u in us:
                    gi, si = group_of(u + 1)
                    if gi not in staging:
                        if gi >= 7:
                            # small late groups get dedicated slots so the
                            # final copies never wait on store completions
                            staging[gi] = spool.tile(
                                [128, gsizes[gi] * W], F32,
                                name=f"staging_l{gi}", bufs=1)
                        else:
                            staging[gi] = spool.tile(
                                [128, gsizes[gi] * W], F32, name="staging")
                    st = staging[gi][:, si * W:(si + 1) * W]
                    if u == -1:
                        nc.scalar.copy(st[64:128, :], tiles[u][64:128, :])
                    elif u == 127:
                        nc.scalar.copy(st[0:64, :], tiles[u][0:64, :])
                    elif u % 2 == 0:
                        nc.vector.tensor_copy(st, tiles[u][:])
                    else:
                        nc.scalar.copy(st, tiles[u][:])
                    if si == gsizes[gi] - 1:
                        emit_stores(gi)

            store_cnt = [0]

            def store_dma(dst, src, late=False, psplit=1):
                # round-robin store engines; sync only joins once its x-load
                # queue has drained, and ACT is kept free late (it does the
                # final PSUM->SBUF copies)
                engs = ([nc.gpsimd, nc.sync] if late
                        else [nc.gpsimd, nc.scalar])
                for ps in range(psplit):
                    p0, p1 = 64 * ps // psplit, 64 * (ps + 1) // psplit
                    eng = engs[store_cnt[0] % len(engs)]
                    store_cnt[0] += 1
                    eng.dma_start(out=dst[p0:p1], in_=src[p0:p1])

            def emit_stores(gi):
                stg = staging.pop(gi)
                s0, s1 = gstart[gi], gstart[gi + 1]
                late = gi >= 4
                sv = stg.rearrange("p (g w) -> p g w", w=W)
                if gi == 0:
                    # row 0 from pair u=-1 -> plane 1, slot 0
                    store_dma(out[1, :, 0:1, :], sv[64:128, 0:1, :])
                # full pairs in this group: slots max(s0,1) .. min(s1,128)-1
                fa, fb = max(s0, 1), min(s1, 128)
                run = 8 if gi < 7 else 4
                va = fa
                while va < fb:
                    vb = min(fb, va + run)
                    G = vb - va
                    store_dma(out[0, :, va:vb, :],
                              sv[0:64, va - s0:va - s0 + G, :], late, 2)
                    store_dma(out[1, :, va:vb, :],
                              sv[64:128, va - s0:va - s0 + G, :], late, 2)
                    va = vb
                if s1 == 129:
                    # row 255 from pair u=127 -> plane 0, slot 128
                    store_dma(out[0, :, NPAIR:NPAIR + 1, :],
                              sv[0:64, 128 - s0:129 - s0, :], True, 2)

            emit_block([-1])
            for b0 in range(0, 128, BLK):
                emit_block(list(range(b0, b0 + BLK)))


_NC_CACHE = None


def _build_nc():
    global _NC_CACHE
    if _NC_CACHE is None:
        nc = bacc.Bacc("TRN2", target_bir_lowering=False, debug=False,
                       num_devices=B)
        with tile.TileContext(nc) as tc:
            _emit_kernel(tc)
        nc.compile()
        _NC_CACHE = nc
    return _NC_CACHE


def _in_maps(x, k_att_logits, dft_weight):
    x = np.asarray(x, dtype=np.float32)
    lg = np.asarray(k_att_logits, dtype=np.float32)
    dw = np.asarray(dft_weight, dtype=np.float32)

    # x -> bf16, parity-split rows, host-inserted zero pad columns
    xp = np.zeros((B, 2, CIN, NPAIR, SLOT), dtype=ml_dtypes.bfloat16)
    xv = x.reshape(B, CIN, NPAIR, 2, W).transpose(0, 3, 1, 2, 4)  # [b,j,c,t,w]
    xp[:, :, :, :, 1:1 + W] = xv.astype(ml_dtypes.bfloat16)

    # host-side gather: dense half-spectrum layout [k, h, w, c], chunk-major
    dftP = dw[:, _INV, :].reshape(KNUM, 2, 96, D2R, 2)   # [k, half, p, w, c]
    dre = dftP[..., 0].transpose(2, 0, 1, 3).reshape(96, 8 * D2R)
    dim = dftP[..., 1].transpose(2, 0, 1, 3).reshape(96, 8 * D2R)
    pk = np.zeros((D2R, _PACKW), dtype=np.float32)
    pk[0:96, _O_DRE:_O_DRE + 8 * D2R] = dre
    pk[0:96, _O_DIM:_O_DIM + 8 * D2R] = dim
    pk[0:96, _O_CB:_O_CB + 2 * D1] = _C2
    pk[0:96, _O_SB:_O_SB + 2 * D1] = _S2

    maps = []
    for b in range(B):
        maps.append({
            "x_in": np.ascontiguousarray(xp[b]),
            "lg_in": lg[b:b + 1],
            "pk_in": pk,
            "ab_in": _AB,
        })
    return maps


def _execute(x, k_att_logits, dft_weight, trace=False, **trace_kwargs):
    nc = _build_nc()
    res = run_bass_kernel_spmd(
        nc, _in_maps(x, k_att_logits, dft_weight),
        core_ids=list(range(B)), trace=trace, **trace_kwargs)
    out = np.empty((B, COUT, H, W), dtype=np.float32)
    for b in range(B):
        dev = res.results[b]["out"]      # [2, COUT, NPAIR+1, W]
        out[b, :, 1::2, :] = dev[0, :, 1:NPAIR + 1, :]
        out[b, :, 0::2, :] = dev[1, :, 0:NPAIR, :]
    return out, res


def kernel(x, k_att_logits, dft_weight):
    out, _ = _execute(x, k_att_logits, dft_weight)
    return out.astype(np.float32)


# revision 40
# speedup vs baseline: 1.0346x; 1.0113x over previous
"""Trainium2 Bass kernel for nn_FDConv (per-sample frequency-domain-synthesized
3x3 grouped conv).

Strategy (data-parallel over batch, 1 sample per NeuronCore):
  - host: permute dft_weight into dense half-spectrum layout (pure gather),
    precompute DFT basis matrices as constants, stage x as bf16 in a padded
    parity-split layout so every load descriptor is one 8KB contiguous run.
  - device per core:
      att = sigmoid(logits)                      (0.5 att scale folded in basis)
      GT  = (att-mixed spectrum)^T . [C | S]     (stage-1 iFFT along axis 0 via
                                                  PE matmuls; kernel-mixing att
                                                  contraction folded in)
      T[which,dx] = A_dx^T . GT_re - B_dx^T . GT_im  (stage-2 irfft + reshape to
                                                  six 128x128 conv weight mats)
      conv: x bf16 with even rows on partitions 0-63 and odd rows on 64-127;
            output row pairs (2u+1, 2u+2) computed as six K=128/M=128/N=256
            matmuls (T1_dx on slot u, T2_dx on slot u+1) accumulated in PSUM.
  - outputs copied PSUM->SBUF (DVE/ACT) and DMA'd back fp32 via SWDGE.
"""

import numpy as np
import ml_dtypes

import concourse.bass as bass
import concourse.bacc as bacc
import concourse.tile as tile
import concourse.mybir as mybir
from concourse.bass_utils import run_bass_kernel_spmd

F32 = mybir.dt.float32
BF16 = mybir.dt.bfloat16

B, CIN, COUT, KS = 8, 64, 64, 3
H, W = 256, 256
KNUM = 4
D1, D2 = COUT * KS, CIN * KS          # 192, 192
D2R = D2 // 2 + 1                     # 97
NF = D1 * D2R                         # 18624

NPAIR = 128          # output row pairs (2u+1, 2u+2), u = -1..127
SLOT = W + 2         # 258: [pad, 256 cols, pad] per row-slot
CHS = 16             # slots per x chunk
NCH = NPAIR // CHS   # 8 chunks
GRP = 16             # pairs per store group
BLK = 4              # pairs per PE block

# constant-pack column offsets (fp32, 97 partitions)
_O_DRE = 0
_O_DIM = 8 * D2R              # 776
_O_CB = 2 * 8 * D2R           # 1552
_O_SB = _O_CB + 2 * D1        # 1936
_PACKW = _O_SB + 2 * D1       # 2320


def _host_constants():
    fh = np.fft.fftfreq(D1)
    fw = np.fft.rfftfreq(D2)
    dist = np.sqrt(fh[:, None] ** 2 + fw[None, :] ** 2)
    idx = np.argsort(dist.ravel(), kind='stable')
    FH = (idx // D2R).astype(np.int64)
    FW = (idx % D2R).astype(np.int64)
    perm = FH * D2R + FW
    inv = np.empty(NF, dtype=np.int64)
    inv[perm] = np.arange(NF)

    hh = np.arange(D1)
    ang = 2.0 * np.pi * np.outer(hh, hh) / D1
    # att scale 2/KNUM = 0.5 folded into the stage-1 basis
    Cb = (np.cos(ang) * (0.5 / D1)).astype(np.float32)
    Sb = (np.sin(ang) * (0.5 / D1)).astype(np.float32)
    C2 = np.concatenate([Cb[:96], Cb[96:]], axis=1)           # [96, 384]
    S2 = np.concatenate([Sb[:96], Sb[96:]], axis=1)           # [96, 384]

    w_ = np.arange(D2R)
    n_ = np.arange(D2)
    alpha = np.full(D2R, 2.0); alpha[0] = 1.0; alpha[-1] = 1.0
    beta = np.full(D2R, 2.0); beta[0] = 0.0; beta[-1] = 0.0
    ang2 = 2.0 * np.pi * np.outer(w_, n_) / D2
    A = (alpha[:, None] * np.cos(ang2) / D2).astype(np.float32)   # [97, 192]
    Bm = (beta[:, None] * np.sin(ang2) / D2).astype(np.float32)
    ab = np.concatenate(
        [A[:, dx::3] for dx in range(3)] + [-Bm[:, dx::3] for dx in range(3)],
        axis=1,
    ).astype(ml_dtypes.bfloat16)                               # [97, 384]
    return inv, C2, S2, ab


_INV, _C2, _S2, _AB = _host_constants()

# (which, dx) order used in the conv weight loop; t_sb index = 2*dx + which
_WSEQ = [(0, 0), (1, 0), (0, 1), (1, 1), (0, 2), (1, 2)]
# valid quadrants (J, r, c0) per `which` (c0 = J - r + 2*which)
_QUADS = {
    0: [(0, 0, 0), (1, 0, 1), (1, 1, 0)],
    1: [(0, 0, 2), (0, 1, 1), (1, 1, 2)],
}
_ZQUAD = {0: (0, 1), 1: (1, 0)}  # zero quadrant (J, r)


def _emit_kernel(tc):
    nc = tc.nc
    from contextlib import ExitStack

    # x: [parity, cin, slot, 258] bf16, host-padded (col 0 and 257 are zeros)
    x_in = nc.dram_tensor("x_in", [2, CIN, NPAIR, SLOT], BF16,
                          kind="ExternalInput").ap()
    lg_in = nc.dram_tensor("lg_in", [1, KNUM], F32, kind="ExternalInput").ap()
    pk_in = nc.dram_tensor("pk_in", [D2R, _PACKW], F32, kind="ExternalInput").ap()
    ab_in = nc.dram_tensor("ab_in", [D2R, 6 * 64], BF16, kind="ExternalInput").ap()
    # device-side output layout: plane r=0 slot s = row 2s-1, plane r=1 slot
    # s = row 2s. Keeps every store descriptor contiguous (multi-KB) per
    # partition; host reassembles rows.
    out = nc.dram_tensor("out", [2, COUT, NPAIR + 1, W], F32,
                         kind="ExternalOutput").ap()

    with ExitStack() as ctx:
        cpool = ctx.enter_context(tc.tile_pool(name="cpool", bufs=1))
        xbpool = ctx.enter_context(tc.tile_pool(name="xbpool", bufs=8))
        spool = ctx.enter_context(tc.tile_pool(name="spool", bufs=3))

        # ---- small input loads (HWDGE sync queue)
        l_sb = cpool.tile([1, KNUM], F32, name="l_sb")
        nc.sync.dma_start(out=l_sb[:], in_=lg_in)
        # sigmoid emitted before the ACT-ring descgen work so it isn't stuck
        # behind blocking dma_start instructions on the ACT stream
        sig_sb = cpool.tile([1, KNUM], F32, name="sig_sb")
        nc.scalar.activation(sig_sb[:], l_sb[:], mybir.ActivationFunctionType.Sigmoid)
        # one DMA lands on ONE SDMA engine (~27 GB/s) with ring depths of ~8
        # (SWDGE) / ~2 (each HWDGE ring), so the latency-critical constant
        # pack is split into 8 row slices spread over all three rings.
        pk_sb = cpool.tile([D2R, _PACKW], F32, name="pk_sb")
        redges = (0, 13, 25, 37, 49, 61, 73, 85, 97)
        pk_engs = [nc.gpsimd, nc.gpsimd, nc.gpsimd, nc.gpsimd,
                   nc.sync, nc.sync, nc.scalar, nc.scalar]
        for k in range(8):
            r0, r1 = redges[k], redges[k + 1]
            pk_engs[k].dma_start(out=pk_sb[r0:r1, :], in_=pk_in[r0:r1, :])
        ab_sb = cpool.tile([D2R, 6 * 64], BF16, name="ab_sb")
        for r0, r1 in ((0, 49), (49, 97)):
            nc.scalar.dma_start(out=ab_sb[r0:r1, :], in_=ab_in[r0:r1, :])
        dre32 = pk_sb[0:96, _O_DRE:_O_DRE + 8 * D2R]
        dim32 = pk_sb[0:96, _O_DIM:_O_DIM + 8 * D2R]
        cb_sb = pk_sb[0:96, _O_CB:_O_CB + 2 * D1]
        sb_sb = pk_sb[0:96, _O_SB:_O_SB + 2 * D1]

        # ---- att broadcast setup (K=1 matmul with ones)
        ones_sb = cpool.tile([1, 128], F32, name="ones_sb")
        nc.vector.memset(ones_sb[:], 1.0)

        # ---- x chunk loads: slot t holds rows (2t, 2t+1). Split into
        # quarter-chunk DMAs alternating SWDGE/HWDGE so many SDMA engines run.
        # Chunks >=2 are delayed behind the dft chain (dep added below) so the
        # small latency-critical loads get the early HBM bandwidth.
        xch = []
        xdmas = []
        early_engs = [nc.gpsimd, nc.sync, nc.gpsimd, nc.scalar,
                      nc.gpsimd, nc.sync, nc.gpsimd, nc.scalar]
        ndma = 0
        for c in range(NCH):
            # chunks hold 17 slots (1-slot overlap) so 2-pair windows never
            # cross a tile boundary; the last chunk has no slot 128
            nsl = CHS + 1 if c + 1 < NCH else CHS
            xb = xbpool.tile([128, (CHS + 1) * SLOT], BF16, name="xb")
            for par in range(2):
                for p0, p1 in ((0, 32), (32, 64)):
                    eng = early_engs[ndma % 8] if c < 2 else nc.gpsimd
                    di = eng.dma_start(
                        out=xb[64 * par + p0: 64 * par + p1, 0:nsl * SLOT],
                        in_=x_in[par, p0:p1, c * CHS:c * CHS + nsl, :])
                    if c >= 2:
                        xdmas.append(di)
                    ndma += 1
            xch.append(xb)

        def slot_rhs(s, dx, npair=1):
            # [128, npair, W] window starting at slot s (npair<=2; both slots
            # live in chunk s//CHS thanks to the 1-slot overlap)
            c, loc = s // CHS, s % CHS
            if npair == 1:
                return xch[c][:, loc * SLOT + dx: loc * SLOT + dx + W]
            v = xch[c].rearrange("p (t s) -> p t s", s=SLOT)
            return v[:, loc:loc + npair, dx:dx + W]

        # ---- bf16 casts of the permuted spectrum
        dre_sb = cpool.tile([96, 8 * D2R], BF16, name="dre_sb")
        nc.vector.tensor_copy(dre_sb[:], dre32)
        dim_sb = cpool.tile([96, 8 * D2R], BF16, name="dim_sb")
        nc.vector.tensor_copy(dim_sb[:], dim32)
        dimneg_sb = cpool.tile([96, 8 * D2R], BF16, name="dimneg_sb")
        nc.vector.tensor_scalar_mul(dimneg_sb[:], dim32, -1.0)

        gtre_sb = cpool.tile([D2R, D1 + 2], BF16, name="gtre_sb")
        gtim_sb = cpool.tile([D2R, D1 + 2], BF16, name="gtim_sb")
        t_sb = [cpool.tile([128, 128], BF16, name=f"t_sb_{i}") for i in range(6)]

        with tc.tile_pool(name="dftps", bufs=1, space="PSUM") as dpool:
            # att broadcast: [128, 4] = ones.T @ sig
            att_ps = dpool.tile([128, KNUM], F32, name="att_ps")
            nc.tensor.matmul(att_ps[:], ones_sb[:], sig_sb[:], start=True, stop=True)
            att_sb = cpool.tile([96, KNUM], F32, name="att_sb")
            nc.vector.tensor_copy(att_sb[:], att_ps[0:96, :])

            # attC/attS on DVE (ACT is busy with descgen; chunk ck = 2k + half)
            attC = cpool.tile([96, 8 * D1], BF16, name="attC")
            attS = cpool.tile([96, 8 * D1], BF16, name="attS")
            for ck in range(8):
                k, half = ck // 2, ck % 2
                nc.vector.tensor_scalar_mul(
                    attC[:, ck * D1:(ck + 1) * D1],
                    cb_sb[:, half * D1:(half + 1) * D1],
                    att_sb[:, k:k + 1])
                nc.vector.tensor_scalar_mul(
                    attS[:, ck * D1:(ck + 1) * D1],
                    sb_sb[:, half * D1:(half + 1) * D1],
                    att_sb[:, k:k + 1])

            # ---- stage 1: GT = F^T . (att*C) etc, contraction over (k, h)
            gtre_ps = dpool.tile([D2R, D1], F32, name="gtre_ps")
            gtim_ps = dpool.tile([D2R, D1], F32, name="gtim_ps")
            for ck in range(8):
                nc.tensor.matmul(
                    gtre_ps[:], dre_sb[:, ck * D2R:(ck + 1) * D2R],
                    attC[:, ck * D1:(ck + 1) * D1],
                    start=(ck == 0), stop=False)
            for ck in range(8):
                nc.tensor.matmul(
                    gtre_ps[:], dimneg_sb[:, ck * D2R:(ck + 1) * D2R],
                    attS[:, ck * D1:(ck + 1) * D1],
                    start=False, stop=(ck == 7))
            for ck in range(8):
                nc.tensor.matmul(
                    gtim_ps[:], dre_sb[:, ck * D2R:(ck + 1) * D2R],
                    attS[:, ck * D1:(ck + 1) * D1],
                    start=(ck == 0), stop=False)
            for ck in range(8):
                nc.tensor.matmul(
                    gtim_ps[:], dim_sb[:, ck * D2R:(ck + 1) * D2R],
                    attC[:, ck * D1:(ck + 1) * D1],
                    start=False, stop=(ck == 7))
            nc.vector.tensor_copy(gtre_sb[:, 0:D1], gtre_ps[:])
            nc.vector.tensor_copy(gtim_sb[:, 0:D1], gtim_ps[:])

            # ---- stage 2: six conv weight matrices T[(ci,j),(co,r)]
            def gview(g, c0):
                return g[:, c0:c0 + D1].rearrange(
                    "w (co th) -> w co th", th=3)[:, :, 0:1]

            t_copy = None
            for i, (which, dx) in enumerate(_WSEQ):
                t_ps = dpool.tile([128, 128], F32, name="t_ps", bufs=3)
                zj, zr = _ZQUAD[which]
                nc.vector.memset(t_ps[64 * zj:64 * zj + 64, 64 * zr:64 * zr + 64], 0.0)
                for (J, r, c0) in _QUADS[which]:
                    o = t_ps[64 * J:64 * J + 64, 64 * r:64 * r + 64]
                    nc.tensor.matmul(o, ab_sb[:, dx * 64:(dx + 1) * 64],
                                     gview(gtre_sb, c0), start=True, stop=False)
                    nc.tensor.matmul(o, ab_sb[:, (3 + dx) * 64:(4 + dx) * 64],
                                     gview(gtim_sb, c0), start=False, stop=True)
                t_copy = nc.vector.tensor_copy(t_sb[2 * dx + which][:], t_ps[:])
        # late x chunks wait for the dft chain to clear the HBM/SDMA path
        for di in xdmas:
            bass._add_dep_helper(di.ins, t_copy.ins,
                                 reason="late x chunks yield HBM to dft chain")

        # ---- conv over row pairs
        # staging groups over pair slots s = u+1 in [0, 129): big early, small
        # at the end so the last stores drain quickly
        gsizes = [16] * 7 + [8, 4, 2, 2, 1]
        gstart = np.cumsum([0] + gsizes).tolist()   # [0,16,...,112,120,124,126,128,129]

        def group_of(s):
            for gi in range(len(gsizes)):
                if s < gstart[gi + 1]:
                    return gi, s - gstart[gi]
            raise AssertionError

        # units: (-1,) special, (0,1), (2,3), ..., (124,125), (126,), (127,)
        units = [(-1,)] + [(u, u + 1) for u in range(0, 126, 2)] + [(126,), (127,)]

        with tc.tile_pool(name="convps", bufs=8, space="PSUM") as cps:
            staging = {}

            def get_staging(gi):
                if gi not in staging:
                    if gi >= 7:
                        # small late groups get dedicated slots so the final
                        # copies never wait on store completions
                        staging[gi] = spool.tile(
                            [128, gsizes[gi] * W], F32,
                            name=f"staging_l{gi}", bufs=1)
                    else:
                        staging[gi] = spool.tile(
                            [128, gsizes[gi] * W], F32, name="staging")
                return staging[gi]

            def unit_mms(un):
                L = []
                for wh, dx in _WSEQ:
                    if wh == 0 and un[0] < 0:
                        continue
                    if wh == 1 and un[0] > 126:
                        continue
                    L.append((wh, dx))
                return L

            def emit_block(uns):
                tiles = {}
                for un in uns:
                    tiles[un] = cps.tile([128, len(un) * W], F32, name="pair_ps")
                plan = {un: unit_mms(un) for un in uns}
                for k, (wh, dx) in enumerate(_WSEQ):
                    for un in uns:
                        if (wh, dx) not in plan[un]:
                            continue
                        i = plan[un].index((wh, dx))
                        rhs = slot_rhs(un[0] + (0 if wh == 0 else 1), dx,
                                       len(un))
                        nc.tensor.matmul(
                            tiles[un][:], t_sb[2 * dx + wh][:], rhs,
                            start=(i == 0), stop=(i == len(plan[un]) - 1),
                            skip_group_check=True)
                for un in uns:
                    for j, u in enumerate(un):
                        gi, si = group_of(u + 1)
                        st = get_staging(gi)[:, si * W:(si + 1) * W]
                        src = tiles[un][:, j * W:(j + 1) * W]
                        if u == -1:
                            nc.scalar.copy(st[64:128, :], src[64:128, :])
                        elif u == 127:
                            nc.scalar.copy(st[0:64, :], src[0:64, :])
                        elif (j == 0 and len(un) == 2 and
                              group_of(un[1] + 1)[0] == gi):
                            # both halves land in the same staging tile: one
                            # wide copy, alternating engines per unit
                            st2 = get_staging(gi)[:, si * W:(si + 2) * W]
                            if (u // 2) % 2 == 0:
                                nc.vector.tensor_copy(st2, tiles[un][:])
                            else:
                                nc.scalar.copy(st2, tiles[un][:])
                            break
                        elif u % 2 == 0:
                            nc.vector.tensor_copy(st, src)
                        else:
                            nc.scalar.copy(st, src)
                    for u in un:
                        gi, si = group_of(u + 1)
                        if si == gsizes[gi] - 1:
                            emit_stores(gi)

            store_cnt = [0]

            def store_dma(dst, src, late=False, psplit=1):
                # round-robin store engines; sync only joins once its x-load
                # queue has drained, and ACT is kept free late (it does the
                # final PSUM->SBUF copies)
                engs = ([nc.gpsimd, nc.sync] if late
                        else [nc.gpsimd, nc.scalar])
                for ps in range(psplit):
                    p0, p1 = 64 * ps // psplit, 64 * (ps + 1) // psplit
                    eng = engs[store_cnt[0] % len(engs)]
                    store_cnt[0] += 1
                    eng.dma_start(out=dst[p0:p1], in_=src[p0:p1])

            def emit_stores(gi):
                stg = staging.pop(gi)
                s0, s1 = gstart[gi], gstart[gi + 1]
                late = gi >= 4
                sv = stg.rearrange("p (g w) -> p g w", w=W)
                if gi == 0:
                    # row 0 from pair u=-1 -> plane 1, slot 0
                    store_dma(out[1, :, 0:1, :], sv[64:128, 0:1, :])
                # full pairs in this group: slots max(s0,1) .. min(s1,128)-1
                fa, fb = max(s0, 1), min(s1, 128)
                run = 8 if gi < 7 else 4
                va = fa
                while va < fb:
                    vb = min(fb, va + run)
                    G = vb - va
                    store_dma(out[0, :, va:vb, :],
                              sv[0:64, va - s0:va - s0 + G, :], late, 2)
                    store_dma(out[1, :, va:vb, :],
                              sv[64:128, va - s0:va - s0 + G, :], late, 2)
                    va = vb
                if s1 == 129:
                    # row 255 from pair u=127 -> plane 0, slot 128
                    store_dma(out[0, :, NPAIR:NPAIR + 1, :],
                              sv[0:64, 128 - s0:129 - s0, :], True, 2)

            # blocks of up to 4 units
            ui = 0
            while ui < len(units):
                emit_block(units[ui:ui + 4])
                ui += 4


_NC_CACHE = None


def _build_nc():
    global _NC_CACHE
    if _NC_CACHE is None:
        nc = bacc.Bacc("TRN2", target_bir_lowering=False, debug=False,
                       num_devices=B)
        with tile.TileContext(nc) as tc:
            _emit_kernel(tc)
        nc.compile()
        _NC_CACHE = nc
    return _NC_CACHE


def _in_maps(x, k_att_logits, dft_weight):
    x = np.asarray(x, dtype=np.float32)
    lg = np.asarray(k_att_logits, dtype=np.float32)
    dw = np.asarray(dft_weight, dtype=np.float32)

    # x -> bf16, parity-split rows, host-inserted zero pad columns
    xp = np.zeros((B, 2, CIN, NPAIR, SLOT), dtype=ml_dtypes.bfloat16)
    xv = x.reshape(B, CIN, NPAIR, 2, W).transpose(0, 3, 1, 2, 4)  # [b,j,c,t,w]
    xp[:, :, :, :, 1:1 + W] = xv.astype(ml_dtypes.bfloat16)

    # host-side gather: dense half-spectrum layout [k, h, w, c], chunk-major
    dftP = dw[:, _INV, :].reshape(KNUM, 2, 96, D2R, 2)   # [k, half, p, w, c]
    dre = dftP[..., 0].transpose(2, 0, 1, 3).reshape(96, 8 * D2R)
    dim = dftP[..., 1].transpose(2, 0, 1, 3).reshape(96, 8 * D2R)
    pk = np.zeros((D2R, _PACKW), dtype=np.float32)
    pk[0:96, _O_DRE:_O_DRE + 8 * D2R] = dre
    pk[0:96, _O_DIM:_O_DIM + 8 * D2R] = dim
    pk[0:96, _O_CB:_O_CB + 2 * D1] = _C2
    pk[0:96, _O_SB:_O_SB + 2 * D1] = _S2

    maps = []
    for b in range(B):
        maps.append({
            "x_in": np.ascontiguousarray(xp[b]),
            "lg_in": lg[b:b + 1],
            "pk_in": pk,
            "ab_in": _AB,
        })
    return maps


def _execute(x, k_att_logits, dft_weight, trace=False, **trace_kwargs):
    nc = _build_nc()
    res = run_bass_kernel_spmd(
        nc, _in_maps(x, k_att_logits, dft_weight),
        core_ids=list(range(B)), trace=trace, **trace_kwargs)
    out = np.empty((B, COUT, H, W), dtype=np.float32)
    for b in range(B):
        dev = res.results[b]["out"]      # [2, COUT, NPAIR+1, W]
        out[b, :, 1::2, :] = dev[0, :, 1:NPAIR + 1, :]
        out[b, :, 0::2, :] = dev[1, :, 0:NPAIR, :]
    return out, res


def kernel(x, k_att_logits, dft_weight):
    out, _ = _execute(x, k_att_logits, dft_weight)
    return out.astype(np.float32)
